# revision 32
# baseline (speedup 1.0000x reference)
"""Multi-head causal attention (B=4, T=2048, D=1024, H=16, HS=64) on 8 TRN2
NeuronCores.

Sharding: batch (4-way) x head-group (2-way).  Core c handles batch c//2 and
heads 8*(c%2) .. 8*(c%2)+7.  Each core computes its 8 heads' attention and the
partial output projection Y_T = sum_h Wo_h^T @ O_T_h; the host sums the two
head-group partials per batch, transposes, and adds the output bias.

Per-core program (all matmuls contract along the partition dim; matmul
datapath in bf16 with fp32 PSUM accumulation, softmax denominator in fp32):
  - x (bf16) is PE-transposed into x^T [d, t]; V^T/Q^T/K^T [e2, t] come from
    matmul(lhsT=W[d, e2], rhs=x^T) with head pairs packed on the PE M axis;
    V^T is re-transposed into V_aug [k, 65] (ones column -> the softmax
    denominator accumulates inside the attn@v matmul for free).
  - S^T blocks [k=128, q=512] = matmul(lhsT=K^T, rhs=Q^T); exp on ScalarE
    (1/sqrt(HS) folded into the activation scale; no max subtraction --
    |scores| <= ~6 so exp cannot overflow); causal mask = upper-tri 0/1
    multiply on the one diagonal sub-block + column offsets on attn@v.
  - O^T_aug [65, q] accumulates over k chunks in PSUM; normalization uses a
    DRAM-bounced partition-broadcast of 1/l (reciprocal_approx_fast).
  - Output projection Y^T[d,q] = sum_pairs matmul(lhsT=Wo[e2,d], rhs=O^T);
    pairs 0-2 are pre-accumulated to SBUF during pair-3's attention.

Engine-level scheduling: Trainium engines execute in order, so emission order
is the schedule.  S^T runs two chunk-pairs ahead of attn@v, and independent
PE work (next head-pair's Q/K projections, out-projection chunks) is emitted
as filler inside the attention stream -- this keeps the PE busy enough that
the HAM clock gate stays at 2.4 GHz instead of demoting to 1.2 GHz.
"""

import numpy as np

B, T, D = 4, 2048, 1024
H, HS = 16, 64
NCORES = 8
NPAIR = 4   # head pairs per core
ND = 8      # 128-wide d chunks
NT = 16     # 128-wide t chunks
NQ = 4      # 512-wide q chunks
NK = 16     # 128-wide k chunks

_CACHE = {}


def _build_program():
    import concourse.bass as bass
    import concourse.tile as tile
    from concourse import bacc, mybir
    from contextlib import ExitStack

    f32 = mybir.dt.float32
    f32r = mybir.dt.float32r
    bf16 = mybir.dt.bfloat16
    Exp = mybir.ActivationFunctionType.Exp

    nc = bacc.Bacc("TRN2", target_bir_lowering=False, debug=False)

    x_d = nc.declare_dram_parameter("x", [T, D], bf16, isOutput=False)
    wq_d = nc.declare_dram_parameter("wq", [NPAIR, 128, ND, 128], bf16, isOutput=False)
    wk_d = nc.declare_dram_parameter("wk", [NPAIR, 128, ND, 128], bf16, isOutput=False)
    wv_d = nc.declare_dram_parameter("wv", [NPAIR, 128, ND, 128], bf16, isOutput=False)
    wo_d = nc.declare_dram_parameter("wo", [128, NPAIR, ND, 128], bf16, isOutput=False)
    tri_d = nc.declare_dram_parameter("tri", [128, 128], bf16, isOutput=False)
    idn_d = nc.declare_dram_parameter("ident", [128, 128], bf16, isOutput=False)
    yt_d = nc.declare_dram_parameter("yt", [D, T], f32, isOutput=True)

    with tile.TileContext(nc) as tc, ExitStack() as top:
        const = top.enter_context(tc.tile_pool(name="const", bufs=1))
        ident_sb = const.tile([128, 128], bf16, name="ident_sb")
        nc.sync.dma_start(out=ident_sb, in_=idn_d[:, :])
        tri_sb = const.tile([128, 128], bf16, name="tri_sb")
        nc.sync.dma_start(out=tri_sb, in_=tri_d[:, :])

        big = top.enter_context(tc.tile_pool(name="big", bufs=1))
        vaug = big.tile([128, 2 * NPAIR, NK, 65], bf16, name="vaug")
        nc.vector.memset(vaug[:, :, :, 64:65], 1.0)

        # PSUM banks: mm 3 + S 2*2 + O 1 = 8
        psM = top.enter_context(tc.tile_pool(name="psM", bufs=3, space="PSUM"))
        psS = top.enter_context(tc.tile_pool(name="psS", bufs=2, space="PSUM"))
        psO = top.enter_context(tc.tile_pool(name="psO", bufs=1, space="PSUM"))
        pw = top.enter_context(tc.tile_pool(name="pw", bufs=2))
        qkp = top.enter_context(tc.tile_pool(name="qkp", bufs=2))
        otn_p = top.enter_context(tc.tile_pool(name="otn_p", bufs=1))
        otn = otn_p.tile([128, NPAIR, T], bf16, name="otn")
        ptp = top.enter_context(tc.tile_pool(name="ptp", bufs=4))
        ocp = top.enter_context(tc.tile_pool(name="ocp", bufs=2))
        rcp = top.enter_context(tc.tile_pool(name="rcp", bufs=2))
        lbp = top.enter_context(tc.tile_pool(name="lbp", bufs=2))
        drp = top.enter_context(tc.tile_pool(name="drp", bufs=4, space="DRAM"))

        def dma_w(wdram, p, kind, pool=None):
            pool = pool or pw
            w_sb = pool.tile([128, ND, 128], bf16, tag="w", name=f"w_{kind}{p}")
            nc.sync.dma_start(out=w_sb, in_=wdram[p])
            return w_sb

        def attn_group(p, hh, j, qt, kt, filler):
            """One (head, q-chunk) attention group with pipelined emission."""
            h = 2 * p + hh
            e0 = hh * 64
            po = psO.tile([65, 512], f32, tag="O", name="po")
            ncc = 4 * (j + 1)
            nm = ncc // 2
            pts = {}

            def off_of(c):
                sub = c - 4 * j
                return sub * 128 if 0 <= sub < 4 else 0

            def emit_s(m):
                ps = psS.tile([128, 2, 512], f32, tag="S", name="ps")
                pt = ptp.tile([128, 2, 512], bf16, tag="pt", name="pt")
                for i in range(2):
                    c = 2 * m + i
                    nc.tensor.matmul(
                        ps[:, i, :],
                        kt[e0:e0 + 64, c * 128:(c + 1) * 128],
                        qt[e0:e0 + 64, j * 512:(j + 1) * 512],
                        start=True,
                        stop=True,
                    )
                nc.scalar.activation(out=pt, in_=ps, func=Exp, scale=0.125)
                for i in range(2):
                    c = 2 * m + i
                    sub = c - 4 * j
                    if 0 <= sub < 4:
                        nc.vector.tensor_mul(
                            pt[:, i, sub * 128:(sub + 1) * 128],
                            pt[:, i, sub * 128:(sub + 1) * 128],
                            tri_sb,
                        )
                pts[m] = pt

            def emit_v(m):
                pt = pts.pop(m)
                for i in range(2):
                    c = 2 * m + i
                    off = off_of(c)
                    nc.tensor.matmul(
                        po[:, off:],
                        vaug[:, h, c, :],
                        pt[:, i, off:],
                        start=(c == 0),
                        stop=(c == ncc - 1),
                    )

            emit_s(0)
            if nm > 1:
                emit_s(1)
            for m in range(nm):
                if m + 2 < nm:
                    emit_s(m + 2)
                filler()
                emit_v(m)

            # normalize: otn[e, q] = O_T[e, q] / l[q]
            oc = ocp.tile([64, 512], f32, tag="oc", name="oc")
            nc.vector.tensor_copy(out=oc, in_=po[0:64, :])
            rl = rcp.tile([1, 512], f32, tag="rl", name="rl")
            nc.vector.tensor_copy(out=rl, in_=po[64:65, :])
            rd = drp.tile([1, 512], f32, tag="rd", name="rd")
            nc.sync.dma_start(out=rd, in_=rl)
            lb = lbp.tile([64, 512], f32, tag="lb", name="lb")
            nc.sync.dma_start(out=lb, in_=rd[0:1, :].partition_broadcast(64))
            nc.vector.reciprocal_approx_fast(lb, lb)
            nc.vector.tensor_mul(
                otn[e0:e0 + 64, p, j * 512:(j + 1) * 512], oc, lb
            )

        with ExitStack() as mid:
            xtp = mid.enter_context(tc.tile_pool(name="xtp", bufs=1))
            xt = xtp.tile([128, ND, T], bf16, name="xt")

            def proj_mms(ps_t4, w_sb, t4, dc_lo, dc_hi):
                for dc in range(dc_lo, dc_hi):
                    nc.tensor.matmul(
                        ps_t4,
                        w_sb[:, dc, :],
                        xt[:, dc, t4 * 512:(t4 + 1) * 512],
                        start=(dc == 0),
                        stop=(dc == ND - 1),
                    )

            def proj_copy(dest_tile, ps_t4, t4):
                nc.vector.tensor_copy(
                    out=dest_tile[:, t4 * 512:(t4 + 1) * 512], in_=ps_t4
                )

            # ---- Phase A: x^T / V-proj / V-transpose, DMA-overlapped -------
            with ExitStack() as ph:
                xa = ph.enter_context(tc.tile_pool(name="xa", bufs=3))
                vts = ph.enter_context(tc.tile_pool(name="vts", bufs=3))
                pwv = ph.enter_context(tc.tile_pool(name="pwv", bufs=4))

                wv_sbs = [None] * NPAIR
                vstash = {}

                def emit_vproj(pv):
                    t4, p = pv // 4, pv % 4
                    ps_t4 = psM.tile([128, 512], f32, tag="mm", name="psv")
                    proj_mms(ps_t4, wv_sbs[p], t4, 0, ND)
                    vt = vts.tile([128, 512], bf16, tag="vt", name="vt")
                    nc.vector.tensor_copy(out=vt, in_=ps_t4)
                    vstash[pv] = vt

                def emit_vtr(pv):
                    t4, p = pv // 4, pv % 4
                    vt = vstash.pop(pv)
                    for hh in range(2):
                        for cl2 in range(2):
                            ptr = psS.tile([128, 2, 1024], bf16, tag="S",
                                           name="ptr_v")
                            for i in range(2):
                                cl = 2 * cl2 + i
                                nc.tensor.transpose(
                                    ptr[:, i, 0:64],
                                    vt[hh * 64:hh * 64 + 64,
                                       cl * 128:(cl + 1) * 128],
                                    ident_sb[hh * 64:hh * 64 + 64,
                                             hh * 64:hh * 64 + 64],
                                )
                            c = 4 * t4 + 2 * cl2
                            nc.vector.tensor_copy(
                                out=vaug[:, 2 * p + hh, c:c + 2, 0:64],
                                in_=ptr[:, :, 0:64],
                            )

                for tt in range(NT):
                    x_sb = xa.tile([128, D], bf16, tag="x_sb", name="x_sb")
                    nc.sync.dma_start(out=x_sb,
                                      in_=x_d[tt * 128:(tt + 1) * 128, :])
                    if tt < NPAIR:
                        wv_sbs[tt] = dma_w(wv_d, tt, "v", pool=pwv)
                    for dc2 in range(ND // 2):
                        ptr = psS.tile([128, 2, 1024], bf16, tag="S",
                                       name="ptr_x")
                        for i in range(2):
                            dc = 2 * dc2 + i
                            nc.tensor.transpose(
                                ptr[:, i, 0:128],
                                x_sb[:, dc * 128:(dc + 1) * 128],
                                ident_sb,
                            )
                        nc.vector.tensor_copy(
                            out=xt[:, 2 * dc2:2 * dc2 + 2,
                                   tt * 128:(tt + 1) * 128],
                            in_=ptr[:, :, 0:128],
                        )
                    if 0 <= tt - 4 < 4 * NPAIR:
                        emit_vproj(tt - 4)
                    if 0 <= tt - 5:
                        emit_vtr(tt - 5)
                for pv in range(NT - 4, 4 * NPAIR):
                    emit_vproj(pv)
                for pv in range(NT - 5, 4 * NPAIR):
                    emit_vtr(pv)

                qt0 = qkp.tile([128, T], bf16, tag="qt", name="qt0")
                kt0 = qkp.tile([128, T], bf16, tag="kt", name="kt0")
                for w_d_, dest, kind in ((wq_d, qt0, "q"), (wk_d, kt0, "k")):
                    w_sb = dma_w(w_d_, 0, kind)
                    for t4 in range(NQ):
                        ps_t4 = psM.tile([128, 512], f32, tag="mm", name="psqk")
                        proj_mms(ps_t4, w_sb, t4, 0, ND)
                        proj_copy(dest, ps_t4, t4)

            # ---- Phase B, pairs 0-2: attention + next-pair Q/K filler ------
            qt_cur, kt_cur = qt0, kt0
            for p in range(NPAIR - 1):
                fill = []
                qt_nxt = qkp.tile([128, T], bf16, tag="qt", name=f"qt{p+1}")
                kt_nxt = qkp.tile([128, T], bf16, tag="kt", name=f"kt{p+1}")
                wq_nxt = dma_w(wq_d, p + 1, "q")
                wk_nxt = dma_w(wk_d, p + 1, "k")
                state = {"ps": None}

                def mk_unit(w_sb, dest, t4, dc_lo, dc_hi, state=state):
                    def emit():
                        if dc_lo == 0:
                            state["ps"] = psM.tile([128, 512], f32, tag="mm",
                                                   name="psf")
                        proj_mms(state["ps"], w_sb, t4, dc_lo, dc_hi)
                        if dc_hi == ND:
                            proj_copy(dest, state["ps"], t4)
                    return emit

                for w_sb, dest in ((wq_nxt, qt_nxt), (wk_nxt, kt_nxt)):
                    for t4 in range(NQ):
                        for dc_lo in range(0, ND, 4):
                            fill.append(mk_unit(w_sb, dest, t4, dc_lo,
                                                dc_lo + 4))

                def filler(fill=fill):
                    if fill:
                        fill.pop(0)()

                for hh in range(2):
                    for j in range(NQ):
                        attn_group(p, hh, j, qt_cur, kt_cur, filler)
                while fill:
                    fill.pop(0)()
                qt_cur, kt_cur = qt_nxt, kt_nxt

        # ---- Tail: pair 3 attention + output projection --------------------
        # head 6 filler: partial out-proj over pairs 0-2 (staged to SBUF);
        # head 7 filler: pair-3 contribution + combine, lagging 2 q-chunks.
        prt_p = top.enter_context(tc.tile_pool(name="prt_p", bufs=1))
        prt = prt_p.tile([128, NQ, ND, 512], f32, name="prt")
        pwo = top.enter_context(tc.tile_pool(name="pwo", bufs=1))
        pyt = top.enter_context(tc.tile_pool(name="pyt", bufs=3))
        wo_sb = pwo.tile([128, NPAIR, ND, 128], bf16, name="wo_sb")
        nc.sync.dma_start(out=wo_sb, in_=wo_d[:, :, :, :])

        def partial_unit(dc, qc):
            def emit():
                py = psM.tile([128, 512], f32, tag="mm", name="pyp")
                for pp in range(NPAIR - 1):
                    nc.tensor.matmul(
                        py,
                        wo_sb[:, pp, dc, :],
                        otn[:, pp, qc * 512:(qc + 1) * 512],
                        start=(pp == 0),
                        stop=(pp == NPAIR - 2),
                    )
                nc.vector.tensor_copy(out=prt[:, qc, dc, :], in_=py)
            return emit

        def final_unit(dc, qc):
            def emit():
                py = psM.tile([128, 512], f32, tag="mm", name="pyf")
                nc.tensor.matmul(
                    py,
                    wo_sb[:, 3, dc, :],
                    otn[:, 3, qc * 512:(qc + 1) * 512],
                    start=True,
                    stop=True,
                )
                yt_sb = pyt.tile([128, 512], f32, tag="yt", name="yt_f")
                nc.vector.tensor_add(yt_sb, prt[:, qc, dc, :], py)
                nc.sync.dma_start(
                    out=yt_d[dc * 128:(dc + 1) * 128,
                             qc * 512:(qc + 1) * 512],
                    in_=yt_sb,
                )
            return emit

        fill3 = [partial_unit(dc, qc) for qc in range(NQ) for dc in range(ND)]
        ffin = []
        done = set()

        def filler3(fill3=fill3):
            if fill3:
                fill3.pop(0)()

        def filler7():
            if ffin:
                ffin.pop(0)()
            elif fill3:
                fill3.pop(0)()

        for j in range(NQ):
            attn_group(3, 0, j, qt_cur, kt_cur, filler3)
        for j in range(NQ):
            if j >= 2:
                qc = j - 2
                for dc in range(ND):
                    ffin.append(final_unit(dc, qc))
                    done.add((dc, qc))
            attn_group(3, 1, j, qt_cur, kt_cur, filler7)
        while fill3:
            fill3.pop(0)()
        while ffin:
            ffin.pop(0)()
        for qc in range(NQ):
            for dc in range(ND):
                if (dc, qc) not in done:
                    final_unit(dc, qc)()

    nc.compile()
    return nc


def _pack_inputs(x, Wq, Wk, Wv, Wo):
    """Per-core input maps. Core c: batch c//2, head group c%2."""
    import ml_dtypes

    tri = np.triu(np.ones((128, 128), np.float32)).astype(ml_dtypes.bfloat16)
    ident = np.eye(128, dtype=np.float32).astype(ml_dtypes.bfloat16)

    def pack_w(W, g):
        # [NPAIR, 128(d_local), ND, 128(e2)]
        out = np.empty((NPAIR, 128, ND, 128), np.float32)
        for p in range(NPAIR):
            h1 = 8 * g + 2 * p
            r = W[[h1, h1 + 1]].transpose(1, 0, 2).reshape(D, 128)  # [d, e2]
            out[p] = r.reshape(ND, 128, 128).transpose(1, 0, 2)
        return np.ascontiguousarray(out).astype(ml_dtypes.bfloat16)

    def pack_wo(Wo, g):
        # [128(e2), NPAIR, ND, 128(d)]
        out = np.empty((128, NPAIR, ND, 128), np.float32)
        for p in range(NPAIR):
            r0 = (8 * g + 2 * p) * 64
            out[:, p] = Wo[r0:r0 + 128].reshape(128, ND, 128)
        return np.ascontiguousarray(out).astype(ml_dtypes.bfloat16)

    packs = {}
    for g in range(2):
        packs[g] = dict(
            wq=pack_w(Wq, g), wk=pack_w(Wk, g), wv=pack_w(Wv, g),
            wo=pack_wo(Wo, g),
        )
    in_maps = []
    for c in range(NCORES):
        b, g = c // 2, c % 2
        m = dict(packs[g])
        m["x"] = np.ascontiguousarray(x[b]).astype(ml_dtypes.bfloat16)
        m["tri"] = tri
        m["ident"] = ident
        in_maps.append(m)
    return in_maps


def kernel(x, Wq, Wk, Wv, Wo, bo):
    from concourse.bass_utils import run_bass_kernel_spmd

    x = np.asarray(x, np.float32)
    Wq, Wk, Wv = (np.asarray(a, np.float32) for a in (Wq, Wk, Wv))
    Wo = np.asarray(Wo, np.float32)
    bo = np.asarray(bo, np.float32)

    if "nc" not in _CACHE:
        _CACHE["nc"] = _build_program()
    nc = _CACHE["nc"]

    in_maps = _pack_inputs(x, Wq, Wk, Wv, Wo)
    res = run_bass_kernel_spmd(nc, in_maps, list(range(NCORES)))
    _CACHE["last_result"] = res

    out = np.empty((B, T, D), np.float32)
    for b in range(B):
        yt = res.results[2 * b]["yt"] + res.results[2 * b + 1]["yt"]
        out[b] = yt.T + bo
    return out


# revision 34
# speedup vs baseline: 1.0297x; 1.0297x over previous
"""Multi-head causal attention (B=4, T=2048, D=1024, H=16, HS=64) on 8 TRN2
NeuronCores.

Sharding: batch (4-way) x head-group (2-way).  Core c handles batch c//2 and
heads 8*(c%2) .. 8*(c%2)+7.  Each core computes its 8 heads' attention and the
partial output projection Y_T = sum_h Wo_h^T @ O_T_h; the host sums the two
head-group partials per batch, transposes, and adds the output bias.

Per-core program (all matmuls contract along the partition dim; matmul
datapath in bf16 with fp32 PSUM accumulation, softmax denominator in fp32):
  - x (bf16) is PE-transposed into x^T [d, t]; V^T/Q^T/K^T [e2, t] come from
    matmul(lhsT=W[d, e2], rhs=x^T) with head pairs packed on the PE M axis;
    V^T is re-transposed into V_aug [k, 65] (ones column -> the softmax
    denominator accumulates inside the attn@v matmul for free).
  - S^T blocks [k=128, q=512] = matmul(lhsT=K^T, rhs=Q^T); exp on ScalarE
    (1/sqrt(HS) folded into the activation scale; no max subtraction --
    |scores| <= ~6 so exp cannot overflow); causal mask = upper-tri 0/1
    multiply on the one diagonal sub-block + column offsets on attn@v.
  - O^T_aug [65, q] accumulates over k chunks in PSUM; normalization uses a
    DRAM-bounced partition-broadcast of 1/l (reciprocal_approx_fast).
  - Output projection Y^T[d,q] = sum_pairs matmul(lhsT=Wo[e2,d], rhs=O^T);
    pairs 0-2 are pre-accumulated to SBUF during pair-3's attention.

Engine-level scheduling: Trainium engines execute in order, so emission order
is the schedule.  S^T runs two chunk-pairs ahead of attn@v, and independent
PE work (next head-pair's Q/K projections, out-projection chunks) is emitted
as filler inside the attention stream -- this keeps the PE busy enough that
the HAM clock gate stays at 2.4 GHz instead of demoting to 1.2 GHz.
"""

import numpy as np

B, T, D = 4, 2048, 1024
H, HS = 16, 64
NCORES = 8
NPAIR = 4   # head pairs per core
ND = 8      # 128-wide d chunks
NT = 16     # 128-wide t chunks
NQ = 4      # 512-wide q chunks
NK = 16     # 128-wide k chunks

_CACHE = {}


def _build_program():
    import concourse.bass as bass
    import concourse.tile as tile
    from concourse import bacc, mybir
    from contextlib import ExitStack

    f32 = mybir.dt.float32
    f32r = mybir.dt.float32r
    bf16 = mybir.dt.bfloat16
    Exp = mybir.ActivationFunctionType.Exp

    nc = bacc.Bacc("TRN2", target_bir_lowering=False, debug=False)

    x_d = nc.declare_dram_parameter("x", [T, D], bf16, isOutput=False)
    wq_d = nc.declare_dram_parameter("wq", [NPAIR, 128, ND, 128], bf16, isOutput=False)
    wk_d = nc.declare_dram_parameter("wk", [NPAIR, 128, ND, 128], bf16, isOutput=False)
    wv_d = nc.declare_dram_parameter("wv", [NPAIR, 128, ND, 128], bf16, isOutput=False)
    wo_d = nc.declare_dram_parameter("wo", [128, NPAIR, ND, 128], bf16, isOutput=False)
    tri_d = nc.declare_dram_parameter("tri", [128, 128], bf16, isOutput=False)
    idn_d = nc.declare_dram_parameter("ident", [128, 128], bf16, isOutput=False)
    yt_d = nc.declare_dram_parameter("yt", [D, T], f32, isOutput=True)

    with tile.TileContext(nc) as tc, ExitStack() as top:
        const = top.enter_context(tc.tile_pool(name="const", bufs=1))
        ident_sb = const.tile([128, 128], bf16, name="ident_sb")
        nc.sync.dma_start(out=ident_sb, in_=idn_d[:, :])
        tri_sb = const.tile([128, 128], bf16, name="tri_sb")
        nc.sync.dma_start(out=tri_sb, in_=tri_d[:, :])

        big = top.enter_context(tc.tile_pool(name="big", bufs=1))
        vaug = big.tile([128, 2 * NPAIR, NK, 65], bf16, name="vaug")
        nc.vector.memset(vaug[:, :, :, 64:65], 1.0)

        # PSUM banks: mm 3 + S 2*2 + O 1 = 8
        psM = top.enter_context(tc.tile_pool(name="psM", bufs=3, space="PSUM"))
        psS = top.enter_context(tc.tile_pool(name="psS", bufs=2, space="PSUM"))
        psO = top.enter_context(tc.tile_pool(name="psO", bufs=1, space="PSUM"))
        pw = top.enter_context(tc.tile_pool(name="pw", bufs=2))
        qkp = top.enter_context(tc.tile_pool(name="qkp", bufs=2))
        otn_p = top.enter_context(tc.tile_pool(name="otn_p", bufs=1))
        otn = otn_p.tile([128, NPAIR, T], bf16, name="otn")
        ptp = top.enter_context(tc.tile_pool(name="ptp", bufs=4))
        ocp = top.enter_context(tc.tile_pool(name="ocp", bufs=2))
        rcp = top.enter_context(tc.tile_pool(name="rcp", bufs=2))
        lbp = top.enter_context(tc.tile_pool(name="lbp", bufs=2))
        drp = top.enter_context(tc.tile_pool(name="drp", bufs=4, space="DRAM"))

        def dma_w(wdram, p, kind, pool=None):
            pool = pool or pw
            w_sb = pool.tile([128, ND, 128], bf16, tag="w", name=f"w_{kind}{p}")
            nc.sync.dma_start(out=w_sb, in_=wdram[p])
            return w_sb

        def attn_group(p, hh, j, qt, kt, filler):
            """One (head, q-chunk) attention group with pipelined emission."""
            h = 2 * p + hh
            e0 = hh * 64
            po = psO.tile([65, 512], f32, tag="O", name="po")
            ncc = 4 * (j + 1)
            nm = ncc // 2
            pts = {}

            def off_of(c):
                sub = c - 4 * j
                return sub * 128 if 0 <= sub < 4 else 0

            def emit_s(m):
                ps = psS.tile([128, 2, 512], f32, tag="S", name="ps")
                pt = ptp.tile([128, 2, 512], bf16, tag="pt", name="pt")
                for i in range(2):
                    c = 2 * m + i
                    nc.tensor.matmul(
                        ps[:, i, :],
                        kt[e0:e0 + 64, c * 128:(c + 1) * 128],
                        qt[e0:e0 + 64, j * 512:(j + 1) * 512],
                        start=True,
                        stop=True,
                    )
                nc.scalar.activation(out=pt, in_=ps, func=Exp, scale=0.125)
                for i in range(2):
                    c = 2 * m + i
                    sub = c - 4 * j
                    if 0 <= sub < 4:
                        nc.vector.tensor_mul(
                            pt[:, i, sub * 128:(sub + 1) * 128],
                            pt[:, i, sub * 128:(sub + 1) * 128],
                            tri_sb,
                        )
                pts[m] = pt

            def emit_v(m):
                pt = pts.pop(m)
                for i in range(2):
                    c = 2 * m + i
                    off = off_of(c)
                    nc.tensor.matmul(
                        po[:, off:],
                        vaug[:, h, c, :],
                        pt[:, i, off:],
                        start=(c == 0),
                        stop=(c == ncc - 1),
                    )

            emit_s(0)
            if nm > 1:
                emit_s(1)
            for m in range(nm):
                if m + 2 < nm:
                    emit_s(m + 2)
                filler()
                emit_v(m)

            # normalize: otn[e, q] = O_T[e, q] / l[q]
            oc = ocp.tile([64, 512], f32, tag="oc", name="oc")
            nc.vector.tensor_copy(out=oc, in_=po[0:64, :])
            rl = rcp.tile([1, 512], f32, tag="rl", name="rl")
            nc.vector.tensor_copy(out=rl, in_=po[64:65, :])
            rd = drp.tile([1, 512], f32, tag="rd", name="rd")
            nc.sync.dma_start(out=rd, in_=rl)
            lb = lbp.tile([64, 512], f32, tag="lb", name="lb")
            nc.sync.dma_start(out=lb, in_=rd[0:1, :].partition_broadcast(64))
            nc.vector.reciprocal_approx_fast(lb, lb)
            nc.vector.tensor_mul(
                otn[e0:e0 + 64, p, j * 512:(j + 1) * 512], oc, lb
            )

        with ExitStack() as mid:
            xtp = mid.enter_context(tc.tile_pool(name="xtp", bufs=1))
            xt = xtp.tile([128, ND, T], bf16, name="xt")

            def proj_mms(ps_t4, w_sb, t4, dc_lo, dc_hi):
                for dc in range(dc_lo, dc_hi):
                    nc.tensor.matmul(
                        ps_t4,
                        w_sb[:, dc, :],
                        xt[:, dc, t4 * 512:(t4 + 1) * 512],
                        start=(dc == 0),
                        stop=(dc == ND - 1),
                    )

            def proj_copy(dest_tile, ps_t4, t4, act=False):
                dst = dest_tile[:, t4 * 512:(t4 + 1) * 512]
                if act:
                    nc.scalar.copy(out=dst, in_=ps_t4)
                else:
                    nc.vector.tensor_copy(out=dst, in_=ps_t4)

            # ---- Phase A: x^T / V-proj / V-transpose, DMA-overlapped -------
            with ExitStack() as ph:
                xa = ph.enter_context(tc.tile_pool(name="xa", bufs=3))
                vts = ph.enter_context(tc.tile_pool(name="vts", bufs=3))
                pwv = ph.enter_context(tc.tile_pool(name="pwv", bufs=4))

                wv_sbs = [None] * NPAIR
                vstash = {}

                def emit_vproj(pv):
                    t4, p = pv // 4, pv % 4
                    ps_t4 = psM.tile([128, 512], f32, tag="mm", name="psv")
                    proj_mms(ps_t4, wv_sbs[p], t4, 0, ND)
                    vt = vts.tile([128, 512], bf16, tag="vt", name="vt")
                    nc.scalar.copy(out=vt, in_=ps_t4)
                    vstash[pv] = vt

                def emit_vtr(pv):
                    t4, p = pv // 4, pv % 4
                    vt = vstash.pop(pv)
                    for hh in range(2):
                        for cl2 in range(2):
                            ptr = psS.tile([128, 2, 1024], bf16, tag="S",
                                           name="ptr_v")
                            for i in range(2):
                                cl = 2 * cl2 + i
                                nc.tensor.transpose(
                                    ptr[:, i, 0:64],
                                    vt[hh * 64:hh * 64 + 64,
                                       cl * 128:(cl + 1) * 128],
                                    ident_sb[hh * 64:hh * 64 + 64,
                                             hh * 64:hh * 64 + 64],
                                )
                            c = 4 * t4 + 2 * cl2
                            nc.scalar.copy(
                                out=vaug[:, 2 * p + hh, c:c + 2, 0:64],
                                in_=ptr[:, :, 0:64],
                            )

                for tt in range(NT):
                    x_sb = xa.tile([128, D], bf16, tag="x_sb", name="x_sb")
                    nc.sync.dma_start(out=x_sb,
                                      in_=x_d[tt * 128:(tt + 1) * 128, :])
                    if tt < NPAIR:
                        wv_sbs[tt] = dma_w(wv_d, tt, "v", pool=pwv)
                    for dc2 in range(ND // 2):
                        ptr = psS.tile([128, 2, 1024], bf16, tag="S",
                                       name="ptr_x")
                        for i in range(2):
                            dc = 2 * dc2 + i
                            nc.tensor.transpose(
                                ptr[:, i, 0:128],
                                x_sb[:, dc * 128:(dc + 1) * 128],
                                ident_sb,
                            )
                        nc.scalar.copy(
                            out=xt[:, 2 * dc2:2 * dc2 + 2,
                                   tt * 128:(tt + 1) * 128],
                            in_=ptr[:, :, 0:128],
                        )
                    if 0 <= tt - 4 < 4 * NPAIR:
                        emit_vproj(tt - 4)
                    if 0 <= tt - 5:
                        emit_vtr(tt - 5)
                for pv in range(NT - 4, 4 * NPAIR):
                    emit_vproj(pv)
                for pv in range(NT - 5, 4 * NPAIR):
                    emit_vtr(pv)

                qt0 = qkp.tile([128, T], bf16, tag="qt", name="qt0")
                kt0 = qkp.tile([128, T], bf16, tag="kt", name="kt0")
                for w_d_, dest, kind in ((wq_d, qt0, "q"), (wk_d, kt0, "k")):
                    w_sb = dma_w(w_d_, 0, kind)
                    for t4 in range(NQ):
                        ps_t4 = psM.tile([128, 512], f32, tag="mm", name="psqk")
                        proj_mms(ps_t4, w_sb, t4, 0, ND)
                        proj_copy(dest, ps_t4, t4, act=True)

            # ---- Phase B, pairs 0-2: attention + next-pair Q/K filler ------
            qt_cur, kt_cur = qt0, kt0
            for p in range(NPAIR - 1):
                fill = []
                qt_nxt = qkp.tile([128, T], bf16, tag="qt", name=f"qt{p+1}")
                kt_nxt = qkp.tile([128, T], bf16, tag="kt", name=f"kt{p+1}")
                wq_nxt = dma_w(wq_d, p + 1, "q")
                wk_nxt = dma_w(wk_d, p + 1, "k")
                state = {"ps": None}

                def mk_unit(w_sb, dest, t4, dc_lo, dc_hi, state=state):
                    def emit():
                        if dc_lo == 0:
                            state["ps"] = psM.tile([128, 512], f32, tag="mm",
                                                   name="psf")
                        proj_mms(state["ps"], w_sb, t4, dc_lo, dc_hi)
                        if dc_hi == ND:
                            proj_copy(dest, state["ps"], t4)
                    return emit

                for w_sb, dest in ((wq_nxt, qt_nxt), (wk_nxt, kt_nxt)):
                    for t4 in range(NQ):
                        for dc_lo in range(0, ND, 4):
                            fill.append(mk_unit(w_sb, dest, t4, dc_lo,
                                                dc_lo + 4))

                def filler(fill=fill):
                    if fill:
                        fill.pop(0)()

                for hh in range(2):
                    for j in range(NQ):
                        attn_group(p, hh, j, qt_cur, kt_cur, filler)
                while fill:
                    fill.pop(0)()
                qt_cur, kt_cur = qt_nxt, kt_nxt

        # ---- Tail: pair 3 attention + output projection --------------------
        # head 6 filler: partial out-proj over pairs 0-2 (staged to SBUF);
        # head 7 filler: pair-3 contribution + combine, lagging 2 q-chunks.
        prt_p = top.enter_context(tc.tile_pool(name="prt_p", bufs=1))
        prt = prt_p.tile([128, NQ, ND, 512], f32, name="prt")
        pwo = top.enter_context(tc.tile_pool(name="pwo", bufs=1))
        pyt = top.enter_context(tc.tile_pool(name="pyt", bufs=3))
        wo_sb = pwo.tile([128, NPAIR, ND, 128], bf16, name="wo_sb")
        nc.sync.dma_start(out=wo_sb, in_=wo_d[:, :, :, :])

        def partial_unit(dc, qc):
            def emit():
                py = psM.tile([128, 512], f32, tag="mm", name="pyp")
                for pp in range(NPAIR - 1):
                    nc.tensor.matmul(
                        py,
                        wo_sb[:, pp, dc, :],
                        otn[:, pp, qc * 512:(qc + 1) * 512],
                        start=(pp == 0),
                        stop=(pp == NPAIR - 2),
                    )
                nc.vector.tensor_copy(out=prt[:, qc, dc, :], in_=py)
            return emit

        def final_unit(dc, qc):
            def emit():
                py = psM.tile([128, 512], f32, tag="mm", name="pyf")
                nc.tensor.matmul(
                    py,
                    wo_sb[:, 3, dc, :],
                    otn[:, 3, qc * 512:(qc + 1) * 512],
                    start=True,
                    stop=True,
                )
                yt_sb = pyt.tile([128, 512], f32, tag="yt", name="yt_f")
                nc.vector.tensor_add(yt_sb, prt[:, qc, dc, :], py)
                nc.sync.dma_start(
                    out=yt_d[dc * 128:(dc + 1) * 128,
                             qc * 512:(qc + 1) * 512],
                    in_=yt_sb,
                )
            return emit

        fill3 = [partial_unit(dc, qc) for qc in range(NQ) for dc in range(ND)]
        ffin = []
        done = set()

        def filler3(fill3=fill3):
            if fill3:
                fill3.pop(0)()

        def filler7():
            if ffin:
                ffin.pop(0)()
            elif fill3:
                fill3.pop(0)()

        for j in range(NQ):
            attn_group(3, 0, j, qt_cur, kt_cur, filler3)
        for j in range(NQ):
            if j >= 2:
                qc = j - 2
                for dc in range(ND):
                    ffin.append(final_unit(dc, qc))
                    done.add((dc, qc))
            attn_group(3, 1, j, qt_cur, kt_cur, filler7)
        while fill3:
            fill3.pop(0)()
        while ffin:
            ffin.pop(0)()
        for qc in range(NQ):
            for dc in range(ND):
                if (dc, qc) not in done:
                    final_unit(dc, qc)()

    nc.compile()
    return nc


def _pack_inputs(x, Wq, Wk, Wv, Wo):
    """Per-core input maps. Core c: batch c//2, head group c%2."""
    import ml_dtypes

    tri = np.triu(np.ones((128, 128), np.float32)).astype(ml_dtypes.bfloat16)
    ident = np.eye(128, dtype=np.float32).astype(ml_dtypes.bfloat16)

    def pack_w(W, g):
        # [NPAIR, 128(d_local), ND, 128(e2)]
        out = np.empty((NPAIR, 128, ND, 128), np.float32)
        for p in range(NPAIR):
            h1 = 8 * g + 2 * p
            r = W[[h1, h1 + 1]].transpose(1, 0, 2).reshape(D, 128)  # [d, e2]
            out[p] = r.reshape(ND, 128, 128).transpose(1, 0, 2)
        return np.ascontiguousarray(out).astype(ml_dtypes.bfloat16)

    def pack_wo(Wo, g):
        # [128(e2), NPAIR, ND, 128(d)]
        out = np.empty((128, NPAIR, ND, 128), np.float32)
        for p in range(NPAIR):
            r0 = (8 * g + 2 * p) * 64
            out[:, p] = Wo[r0:r0 + 128].reshape(128, ND, 128)
        return np.ascontiguousarray(out).astype(ml_dtypes.bfloat16)

    packs = {}
    for g in range(2):
        packs[g] = dict(
            wq=pack_w(Wq, g), wk=pack_w(Wk, g), wv=pack_w(Wv, g),
            wo=pack_wo(Wo, g),
        )
    in_maps = []
    for c in range(NCORES):
        b, g = c // 2, c % 2
        m = dict(packs[g])
        m["x"] = np.ascontiguousarray(x[b]).astype(ml_dtypes.bfloat16)
        m["tri"] = tri
        m["ident"] = ident
        in_maps.append(m)
    return in_maps


def kernel(x, Wq, Wk, Wv, Wo, bo):
    from concourse.bass_utils import run_bass_kernel_spmd

    x = np.asarray(x, np.float32)
    Wq, Wk, Wv = (np.asarray(a, np.float32) for a in (Wq, Wk, Wv))
    Wo = np.asarray(Wo, np.float32)
    bo = np.asarray(bo, np.float32)

    if "nc" not in _CACHE:
        _CACHE["nc"] = _build_program()
    nc = _CACHE["nc"]

    in_maps = _pack_inputs(x, Wq, Wk, Wv, Wo)
    res = run_bass_kernel_spmd(nc, in_maps, list(range(NCORES)))
    _CACHE["last_result"] = res

    out = np.empty((B, T, D), np.float32)
    for b in range(B):
        yt = res.results[2 * b]["yt"] + res.results[2 * b + 1]["yt"]
        out[b] = yt.T + bo
    return out
